# revision 7
# baseline (speedup 1.0000x reference)
"""AttentionBlock Trainium2 kernel (v4: fp8 DoubleRow QKV-proj + AV, gap-free
pipeline).

Data-parallel: one batch element per NeuronCore (8 cores, no collectives).

Per core, with xr = x[b] viewed as [C, S] (C=512 channels, S=1024 tokens):
    QT = wq^T @ xr + bq   -> [D, S]  (d on partitions; head h = rows 64h..64h+63)
    KT = wk^T @ xr + bk   -> [D, S]
    V  = xr^T @ wv + bv   -> [S, D]  (tokens on partitions)
    per head h: ET[j, i] = KT_h^T . QT_h            (keys j on psum partitions)
                E = exp(ET / sqrt(C) - G)           (G cancels in O'/Z; keeps E
                                                     inside e5m2 range, e <= 11)
                O'T[d, i] = sum_j V[j, d] E[j, i];  Z[i] = sum_j E[j, i]
                OT[d, i] = O'T[d, i] / Z[i]
    y = wo^T @ OT + bo + xr   -> [C, S]

PE modes (measured: fp8 DoubleRow streams 2 fp8/cycle = 2x fp16 FLOP rate;
216ns per N=512 matmul either way):
  - QKV projections: DoubleRow e4m3. x8/w8 in [p, j(chunk of 256), i(k-tile),
    .] layout; contraction 512 = 2 chunks x (128 partitions x 2 k-tiles).
  - energy: fp16 K=64 head pairs at partition offsets 0/64 (64-contraction is
    moving-stream-bound; fp8 gains nothing).
  - AV: DoubleRow, et e5m2 x Vp e4m3 (mixed dtypes verified). Key chunks in
    pairs; Vp cols = [ones(64)|V(64)] so Z lands on psum rows 0..63.
  - final projection: fp16 (fp8 here measured ~1.3e-2 rel err alone - too hot).

exp: ScalarE table exp -> e5m2 (scale=1/sqrt(C), bias=-G) for most key-chunk
units; DVE Schraudolph e5m2 bit-trick u8(x*A+B) for ATTN_DVE_EXP_UNITS
(default 3,4,5). Per-element exp error cancels in the O'/Z ratio.

Pipeline notes (v4 — every PE gap costs ~3us of half-clock pstate ramp):
  - AV psum alternates between two 2-bank slots (A even halves, B odd): the
    next half's AV start never WARs against the previous half's norm reads.
  - AV runs per pair, deferred one pair; the last pair of a half crosses the
    seam and is flushed after the next half's jc0+jc1 energies (~900ns cover
    for its exp). The norm flushes right after that flush.
  - QK DoubleRow for t+1 is hoisted at jc5/jc6/jc7 of one half of t, into the
    slot the previous norm just finished reading (~jc5.8). Its psum->fp16
    copies run on ACT (Identity + per-partition bias), emitted before that
    half's trailing ACT exps.
  - DMA: sync [x8, xb cc0, cc1]; scalar [bq, bk, wq8/wk8 t0-cols, rest] (all
    early so the ACT engine is clear for exp); gpsimd [bv, wv8, xb cc2, cc3,
    bo, wo]. V-proj feeds from x8 (fp8) so nothing big is needed before ~12us.
"""

import math
import os

import numpy as np

B = 8
C = 512
S = 1024  # 32*32 tokens
NH = 8
HD = 64
P = 128
CC = C // P  # 4 contraction chunks of 128
NI = 2  # S split into 2 chunks of 512 for matmul free dim
SC = S // P  # 8 key chunks of 128
NP = SC // 2  # 4 key-chunk pairs for the DoubleRow AV

G_OFF = float(os.environ.get("ATTN_G", "1.5"))
SCHRAUD_C = float(os.environ.get("ATTN_SCHRAUD_C", "0.5"))
N_WARM = int(os.environ.get("ATTN_WARM", "8"))
_DVE_UNITS = tuple(
    int(u) for u in os.environ.get("ATTN_DVE_EXP_UNITS", "3,4,5").split(",") if u != ""
)
QK_COPY = os.environ.get("ATTN_QK_COPY", "act")


def _emit(nc, tc, mybir, aps):
    import contextlib

    F32 = mybir.dt.float32
    F16 = mybir.dt.float16
    E4 = mybir.dt.float8e4
    E5 = mybir.dt.float8e5
    U8 = mybir.dt.uint8
    U16 = mybir.dt.uint16
    DR = mybir.MatmulPerfMode.DoubleRow
    MULT = mybir.AluOpType.mult
    ADD = mybir.AluOpType.add
    EXP = mybir.ActivationFunctionType.Exp
    IDENT = mybir.ActivationFunctionType.Identity
    softmax_scale = 1.0 / math.sqrt(C)
    ONE_E4 = 0x38  # 1.0 in e4m3

    SCH_A = softmax_scale * 4.0 / math.log(2.0)
    SCH_B = 15.0 * 4.0 - SCHRAUD_C - G_OFF * 4.0 / math.log(2.0)

    xb, x8, wq8, wk8, wv8, bq, bk, bv, wo, bo, y = (
        aps[k]
        for k in ("xb", "x8", "wq8", "wk8", "wv8", "bq", "bk", "bv", "wo", "bo", "y")
    )
    xb_r = xb.rearrange("(cc p) s -> p cc s", p=P)
    y_r = y.rearrange("(cc p) s -> p cc s", p=P)
    x8_r = x8.rearrange("(j i p) s -> p j i s", p=P, i=2)
    wq8_r = wq8.rearrange("(j i p) d -> p j i d", p=P, i=2)
    wk8_r = wk8.rearrange("(j i p) d -> p j i d", p=P, i=2)
    wv8_r = wv8.rearrange("(j i p) d -> p j i d", p=P, i=2)
    wo_r = wo.rearrange("(dc p) c -> p dc c", p=P)
    bq_r = bq.rearrange("(dc p) -> p dc", p=P)
    bk_r = bk.rearrange("(dc p) -> p dc", p=P)
    bo_r = bo.rearrange("(cc p) -> p cc", p=P)

    with contextlib.ExitStack() as ctx:
        singles = ctx.enter_context(tc.tile_pool(name="singles", bufs=1))
        qkpool = ctx.enter_context(tc.tile_pool(name="qk", bufs=2))
        etpool = ctx.enter_context(tc.tile_pool(name="et", bufs=4))
        rbpool = ctx.enter_context(tc.tile_pool(name="rb", bufs=4))
        tmppool = ctx.enter_context(tc.tile_pool(name="tmp", bufs=4))
        # PSUM (8 banks): energy 2x2 (rotation), slot A 2, slot B 2.
        pse = ctx.enter_context(tc.tile_pool(name="pse", bufs=2, space="PSUM"))
        psA = ctx.enter_context(tc.tile_pool(name="psA", bufs=1, space="PSUM"))
        psB = ctx.enter_context(tc.tile_pool(name="psB", bufs=1, space="PSUM"))

        def av_pool(h):
            return psA if h % 2 == 0 else psB

        def aux_pool(h):
            return psB if h % 2 == 0 else psA

        # ---- input DMAs ----
        xb_sb = singles.tile([P, CC, S], F16)
        x8_sb = singles.tile([P, 2, 2, S], E4)
        wq8_sb = singles.tile([P, 2, 2, C], E4)
        wk8_sb = singles.tile([P, 2, 2, C], E4)
        wv8_sb = singles.tile([P, 2, 2, C], E4)
        bq_sb = singles.tile([P, CC], F32)
        bk_sb = singles.tile([P, CC], F32)
        bo_sb = singles.tile([P, CC], F32)
        bv_sb = singles.tile([1, C], F16)
        wo_sb = singles.tile([P, CC, C], F16)

        nc.sync.dma_start(out=x8_sb, in_=x8_r)
        nc.sync.dma_start(out=xb_sb[:, 0], in_=xb_r[:, 0])
        nc.sync.dma_start(out=xb_sb[:, 1], in_=xb_r[:, 1])
        nc.scalar.dma_start(out=bq_sb, in_=bq_r)
        nc.scalar.dma_start(out=bk_sb, in_=bk_r)
        nc.scalar.dma_start(out=wq8_sb[:, :, :, 0:128], in_=wq8_r[:, :, :, 0:128])
        nc.scalar.dma_start(out=wk8_sb[:, :, :, 0:128], in_=wk8_r[:, :, :, 0:128])
        nc.scalar.dma_start(out=wq8_sb[:, :, :, 128:512], in_=wq8_r[:, :, :, 128:512])
        nc.scalar.dma_start(out=wk8_sb[:, :, :, 128:512], in_=wk8_r[:, :, :, 128:512])
        nc.gpsimd.dma_start(out=bv_sb, in_=bv[None, :])
        nc.gpsimd.dma_start(out=wv8_sb, in_=wv8_r)
        nc.gpsimd.dma_start(out=xb_sb[:, 2], in_=xb_r[:, 2])
        nc.gpsimd.dma_start(out=xb_sb[:, 3], in_=xb_r[:, 3])
        nc.gpsimd.dma_start(out=bo_sb, in_=bo_r)
        nc.gpsimd.dma_start(out=wo_sb, in_=wo_r)

        # V' stationary for the DoubleRow AV: per (pair, head, slot):
        # 128 cols = [ones(64) | V(64)] e4m3.
        Vp8 = singles.tile([P, NP, NH, 2, P], E4)
        for pp in range(NP):
            nc.gpsimd.memset(Vp8[:, pp, :, :, 0:64].bitcast(U8), ONE_E4)
        bv_rep = singles.tile([P, C], F16)
        nc.gpsimd.partition_broadcast(bv_rep, bv_sb, channels=P)

        negg_sb = singles.tile([P, 1], F32)
        nc.vector.memset(negg_sb, -G_OFF)

        warm = singles.tile([P, 512], F16)
        nc.vector.memset(warm.bitcast(U16), 0)
        ps_w = psB.tile([P, 2, 512], F32, tag="b")
        for _ in range(N_WARM):
            nc.tensor.matmul(ps_w[:, 0], warm[:, 0:128], warm)

        OTs = [singles.tile([P, S], F16, tag=f"ot{t}", name=f"ot{t}") for t in range(CC)]

        def emit_v_projection_chunk(sc):
            # V[s, d] via DoubleRow from x8/wv8; cast+bias into Vp8 value cols.
            ps_v = pse.tile([P, 2, 512], F32, tag="e")
            for j in range(2):
                nc.tensor.matmul(
                    ps_v[:, 0],
                    x8_sb[:, j, :, sc * P : (sc + 1) * P],
                    wv8_sb[:, j],
                    start=(j == 0),
                    stop=(j == 1),
                    perf_mode=DR,
                )
            psv_r = ps_v[:, 0].rearrange("p (h d) -> p h d", h=NH)
            bv_r2 = bv_rep.rearrange("p (h d) -> p h d", h=NH)
            nc.vector.tensor_tensor(
                Vp8[:, sc // 2, :, sc % 2, 64:128], psv_r, bv_r2, ADD
            )

        pending_norm = [None]
        pending_av = []

        def flush_av(depth=0):
            while len(pending_av) > depth:
                pending_av.pop(0)()

        def flush_norm():
            if pending_norm[0] is not None:
                pending_norm[0]()
                pending_norm[0] = None

        qk_tiles = {}

        def qk_copy(dst, src, bias_ap):
            if QK_COPY == "act":
                nc.scalar.activation(out=dst, in_=src, func=IDENT, bias=bias_ap, scale=1.0)
            else:
                nc.vector.tensor_scalar_add(dst, src, bias_ap)

        def make_qk_steps(tn, h):
            # Hoisted DoubleRow QK for pair tn in the aux slot of half h:
            # [mms(0)] @jc5, [bias(0)+mms(1)] @jc6, [bias(1)] @jc7.
            qt = qkpool.tile([P, S], F16, tag="qt", name=f"qt{tn}")
            kt = qkpool.tile([P, S], F16, tag="kt", name=f"kt{tn}")
            qk_tiles[tn] = (qt, kt)
            ps_list = []
            dsl = slice(tn * P, (tn + 1) * P)

            def mms(i):
                sl = slice(i * 512, (i + 1) * 512)
                ps_p = aux_pool(h).tile([P, 2, 512], F32, tag="a" if h % 2 else "b")
                ps_list.append(ps_p)
                for j in range(2):
                    nc.tensor.matmul(
                        ps_p[:, 0], wq8_sb[:, j, :, dsl], x8_sb[:, j, :, sl],
                        start=(j == 0), stop=(j == 1), perf_mode=DR,
                    )
                    nc.tensor.matmul(
                        ps_p[:, 1], wk8_sb[:, j, :, dsl], x8_sb[:, j, :, sl],
                        start=(j == 0), stop=(j == 1), perf_mode=DR,
                    )

            def bias(i):
                sl = slice(i * 512, (i + 1) * 512)
                qk_copy(qt[:, sl], ps_list[i][:, 0], bq_sb[:, tn : tn + 1])
                qk_copy(kt[:, sl], ps_list[i][:, 1], bk_sb[:, tn : tn + 1])

            return [
                lambda: mms(0),
                lambda: (bias(0), mms(1)),
                lambda: bias(1),
            ]

        # ---- t=0 QK (no hoist cover yet; uses slot B while warm drains) ----
        for step in make_qk_steps(0, 1):
            step()

        for t in range(CC):
            qt, kt = qk_tiles.pop(t)
            h0, h1 = 2 * t, 2 * t + 1
            for i in range(NI):
                h = 2 * t + i
                sl = slice(i * 512, (i + 1) * 512)
                ps_av = None
                et_pair = None
                for jc in range(SC):
                    ih = 1 if t == 0 else 0
                    if i == ih and t < CC - 1:
                        if jc == 5:
                            qk_steps = make_qk_steps(t + 1, h)
                            qk_steps[0]()
                        elif jc == 6:
                            qk_steps[1]()
                        elif jc == 7:
                            qk_steps[2]()
                    k0 = kt[0:64, jc * P : (jc + 1) * P]
                    k1 = kt[64:128, jc * P : (jc + 1) * P]
                    ps_e = pse.tile([P, 2, 512], F32, tag="e")  # head-major
                    nc.tensor.matmul(ps_e[:, 0], k0, qt[0:64, sl])
                    nc.tensor.matmul(ps_e[:, 1], k1, qt[64:128, sl])
                    if t == 0 and i == 0:
                        for sc in {2: (0,), 3: (1,), 4: (2,), 5: (3,),
                                   6: (4, 5), 7: (6, 7)}.get(jc, ()):
                            emit_v_projection_chunk(sc)
                    if jc % 2 == 0:
                        et_pair = etpool.tile([P, 2, 2, 512], E5, tag="et")
                    et_out = et_pair[:, jc % 2]
                    if jc in _DVE_UNITS:
                        nc.vector.tensor_scalar(
                            et_out.bitcast(U8), ps_e, SCH_A, SCH_B, MULT, ADD
                        )
                    else:
                        nc.scalar.activation(
                            out=et_out, in_=ps_e, func=EXP,
                            scale=softmax_scale, bias=negg_sb[:, 0:1],
                        )
                    if jc == 0:
                        # allocate this half's AV accumulator in its slot; the
                        # previous half's last AV pair is still pending (its
                        # exp needs jc0+jc1 energy cover).
                        ps_av = av_pool(h).tile(
                            [P, 2, 512], F32, tag="a" if h % 2 == 0 else "b"
                        )
                    if jc == 1:
                        flush_av()   # previous half's last pair
                        flush_norm()  # previous half's norm (slot now quiet)
                    if jc % 2 == 1:
                        flush_av(depth=1)
                        pp = jc // 2

                        def av(ps_av=ps_av, pp=pp, et=et_pair, h0=h0, h1=h1):
                            nc.tensor.matmul(
                                ps_av[:, 0], Vp8[:, pp, h0], et[:, :, 0],
                                start=(pp == 0), stop=(pp == NP - 1),
                                perf_mode=DR,
                            )
                            nc.tensor.matmul(
                                ps_av[:, 1], Vp8[:, pp, h1], et[:, :, 1],
                                start=(pp == 0), stop=(pp == NP - 1),
                                perf_mode=DR,
                            )

                        pending_av.append(av)

                def norm(t=t, sl=sl, ps_av=ps_av):
                    rb = rbpool.tile([64, 2, 512], F32, tag="rb")
                    nc.vector.reciprocal_approx_fast(out=rb, in_=ps_av[0:64])
                    nc.vector.tensor_tensor(
                        OTs[t][0:64, sl], ps_av[64:128, 0], rb[:, 0], MULT
                    )
                    nc.vector.tensor_tensor(
                        OTs[t][64:128, sl], ps_av[64:128, 1], rb[:, 1], MULT
                    )

                pending_norm[0] = norm

        # ---- final projection + bias + residual (fp16) ----
        # cc0 accumulates in slot A (the only slot free at tail start); the
        # first fmms cover the last AV pair's exp, then the last norm flushes;
        # cc3 lives in the slot that norm reads and is emitted after 18 cover
        # matmuls (as v2). cc1/cc2 ride the pse rotation (freed as the
        # trailing exps drain).
        ps_fs = [
            psA.tile([P, 2, 512], F32, tag="a", name="psf0"),
            pse.tile([P, 2, 512], F32, tag="e", name="psf1"),
            pse.tile([P, 2, 512], F32, tag="e", name="psf2"),
            psB.tile([P, 2, 512], F32, tag="b", name="psf3"),
        ]

        def fmm(dc, cc, start, stop):
            wo_sl = wo_sb[:, dc, cc * P : (cc + 1) * P]
            for i in range(NI):
                sl = slice(i * 512, (i + 1) * 512)
                nc.tensor.matmul(
                    ps_fs[cc][:, i], wo_sl, OTs[dc][:, sl], start=start, stop=stop,
                )

        fmm(0, 0, True, False)
        flush_av()
        fmm(1, 0, False, False)
        flush_norm()
        fmm(2, 0, False, False)
        for cc in range(1, CC - 1):
            for dc in range(CC - 1):
                fmm(dc, cc, dc == 0, False)
        for dc in range(CC - 1):
            fmm(dc, 3, dc == 0, False)
        out_q = [nc.sync, nc.scalar, nc.gpsimd]
        for cc in range(CC):
            fmm(CC - 1, cc, False, True)
            for i in range(NI):
                sl = slice(i * 512, (i + 1) * 512)
                tmp = tmppool.tile([P, 512], F16, tag="tmp")
                nc.vector.scalar_tensor_tensor(
                    out=tmp,
                    in0=ps_fs[cc][:, i],
                    scalar=bo_sb[:, cc : cc + 1],
                    in1=xb_sb[:, cc, sl],
                    op0=ADD,
                    op1=ADD,
                )
                out_q[(2 * cc + i) % 3].dma_start(out=y_r[:, cc, sl], in_=tmp)


_NC_CACHE = {}


def _build():
    key = (G_OFF, SCHRAUD_C, N_WARM, _DVE_UNITS, QK_COPY)
    if key in _NC_CACHE:
        return _NC_CACHE[key]
    import concourse.bacc as bacc
    import concourse.mybir as mybir
    import concourse.tile as tile

    F32 = mybir.dt.float32
    F16 = mybir.dt.float16
    E4 = mybir.dt.float8e4
    nc = bacc.Bacc("TRN2", target_bir_lowering=False, debug=False)
    aps = {}
    aps["xb"] = nc.dram_tensor("xb", (C, S), F16, kind="ExternalInput").ap()
    aps["x8"] = nc.dram_tensor("x8", (C, S), E4, kind="ExternalInput").ap()
    aps["wq8"] = nc.dram_tensor("wq8", (C, C), E4, kind="ExternalInput").ap()
    aps["wk8"] = nc.dram_tensor("wk8", (C, C), E4, kind="ExternalInput").ap()
    aps["wv8"] = nc.dram_tensor("wv8", (C, C), E4, kind="ExternalInput").ap()
    aps["wo"] = nc.dram_tensor("wo", (C, C), F16, kind="ExternalInput").ap()
    for name in ("bq", "bk", "bo"):
        aps[name] = nc.dram_tensor(name, (C,), F32, kind="ExternalInput").ap()
    aps["bv"] = nc.dram_tensor("bv", (C,), F16, kind="ExternalInput").ap()
    aps["y"] = nc.dram_tensor("y", (C, S), F16, kind="ExternalOutput").ap()
    with tile.TileContext(nc) as tc:
        _emit(nc, tc, mybir, aps)
    nc.compile()
    _NC_CACHE[key] = nc
    return nc


def prepare_in_maps(x, wq, bq, wk, bk, wv, bv, wo, bo):
    """Host-side prep: cast to f16/e4m3, shard x per core."""
    import ml_dtypes

    E4 = ml_dtypes.float8_e4m3
    x = np.asarray(x, dtype=np.float32).reshape(B, C, S)
    weights = {
        "wq8": np.ascontiguousarray(np.asarray(wq, dtype=np.float32).astype(E4)),
        "bq": np.ascontiguousarray(np.asarray(bq, dtype=np.float32)),
        "wk8": np.ascontiguousarray(np.asarray(wk, dtype=np.float32).astype(E4)),
        "bk": np.ascontiguousarray(np.asarray(bk, dtype=np.float32)),
        "wv8": np.ascontiguousarray(np.asarray(wv, dtype=np.float32).astype(E4)),
        "bv": np.ascontiguousarray(np.asarray(bv, dtype=np.float16)),
        "wo": np.ascontiguousarray(np.asarray(wo, dtype=np.float16)),
        "bo": np.ascontiguousarray(np.asarray(bo, dtype=np.float32)),
    }
    return [
        {
            "xb": np.ascontiguousarray(x[b].astype(np.float16)),
            "x8": np.ascontiguousarray(x[b].astype(E4)),
            **weights,
        }
        for b in range(B)
    ]


def kernel(x, wq, bq, wk, bk, wv, bv, wo, bo):
    from concourse import bass_utils

    nc = _build()
    in_maps = prepare_in_maps(x, wq, bq, wk, bk, wv, bv, wo, bo)
    res = bass_utils.run_bass_kernel_spmd(nc, in_maps, core_ids=list(range(B)))
    out = np.stack([r["y"].astype(np.float32) for r in res.results])
    return out.reshape(B, C, 32, 32)


# revision 8
# speedup vs baseline: 1.1064x; 1.1064x over previous
"""AttentionBlock Trainium2 kernel (v5).

Data-parallel: one batch element per NeuronCore (8 cores, no collectives).

Per core, with xr = x[b] viewed as [C, S] (C=512 channels, S=1024 tokens):
    QT = wq^T @ xr + bq   -> [D, S]  (d on partitions; head h = rows 64h..64h+63)
    KT = wk^T @ xr + bk   -> [D, S]
    V  = xr^T @ wv + bv   -> [S, D]  (tokens on partitions)
    per head h: ET[j, i] = KT_h^T . QT_h            (keys j on psum partitions)
                E = exp(ET / sqrt(C) - G)           (G cancels in O'/Z; keeps E
                                                     inside e5m2 range, e <= 11)
                O'T[d, i] = sum_j V[j, d] E[j, i];  Z[i] = sum_j E[j, i]
                OT[d, i] = O'T[d, i] / Z[i]
    y = wo^T @ OT + bo + xr   -> [C, S]

PE modes (measured: fp8 DoubleRow = 2x fp16 FLOP rate; 216ns per N=512 matmul
either way; issue rate is one column/cycle regardless of contraction depth):
  - QKV projections + AV: DoubleRow fp8 (e4m3 operands; et in e5m2 - exp can
    reach 6e4, beyond e4m3 range - mixed-dtype DR verified on HW).
  - energy: fp16 K=64 head pairs at partition offsets 0/64.
  - final projection: fp16 (fp8 there alone measured ~1.3e-2 rel err).

The whole schedule is paced by energy-psum drain: 2 tiles x 2 banks rotate,
each must be drained by exp (~1.1us) before the energy 2 chunks later can
start. Per half (t, i): PE 5.2us, ACT 5 exps 5.6us, DVE 3 exps + norm 6.3us.
v5 structure to keep that pipe gap-free (every PE gap costs ~3us of
half-clock pstate ramp):
  - ALL QT/KT are computed in a prologue (fp8 DR + psum->fp16 copies on
    ACT(q)/DVE(k)), overlapping the input DMAs. No mid-stream hoisting.
  - exp engine per key-chunk: jc 1,3,5 on DVE (Schraudolph e5m2), rest on
    ScalarE table exp. Alternating drains keep up with the 0.65us/jc PE pace.
  - AV psum alternates slots A/B per half; the previous half's last AV pair
    crosses the seam (flushed after jc0+jc1 energies cover its exp).
  - norm(h) is flushed at h+1's jc6 - BEHIND h+1's DVE exps - so it never
    delays the rotation drains; its slot isn't rewritten until AV(h+2) p0
    (pops at h+2 jc5), which clears comfortably. The last two norms flush
    early/at the tail (old style) so the tail isn't DVE-serialized.
  - V projection: fp8 DR from x8/wv8, staggered through (t0,i0); its Vp8
    copies (+bv, ->e4m3) ride DVE.
  - DMA: sync [x8, xb cc0, cc1]; scalar [bq, bk, wq8/wk8 interleaved per
    head-pair]; gpsimd [bv, wv8, bo, wo, xb cc2, cc3]. Weights for pair t
    land just before the prologue needs them.
"""

import math
import os

import numpy as np

B = 8
C = 512
S = 1024  # 32*32 tokens
NH = 8
HD = 64
P = 128
CC = C // P  # 4 contraction chunks of 128
NI = 2  # S split into 2 chunks of 512 for matmul free dim
SC = S // P  # 8 key chunks of 128
NP = SC // 2  # 4 key-chunk pairs for the DoubleRow AV

G_OFF = float(os.environ.get("ATTN_G", "1.5"))
SCHRAUD_C = float(os.environ.get("ATTN_SCHRAUD_C", "0.5"))
N_WARM = int(os.environ.get("ATTN_WARM", "10"))
_DVE_UNITS = tuple(
    int(u) for u in os.environ.get("ATTN_DVE_EXP_UNITS", "1,3,5").split(",") if u != ""
)


def _emit(nc, tc, mybir, aps):
    import contextlib

    F32 = mybir.dt.float32
    F16 = mybir.dt.float16
    E4 = mybir.dt.float8e4
    E5 = mybir.dt.float8e5
    U8 = mybir.dt.uint8
    U16 = mybir.dt.uint16
    DR = mybir.MatmulPerfMode.DoubleRow
    MULT = mybir.AluOpType.mult
    ADD = mybir.AluOpType.add
    EXP = mybir.ActivationFunctionType.Exp
    IDENT = mybir.ActivationFunctionType.Identity
    softmax_scale = 1.0 / math.sqrt(C)
    ONE_E4 = 0x38  # 1.0 in e4m3

    SCH_A = softmax_scale * 4.0 / math.log(2.0)
    SCH_B = 15.0 * 4.0 - SCHRAUD_C - G_OFF * 4.0 / math.log(2.0)

    xb, x8, wq8, wk8, wv8, bq, bk, bv, wo, bo, y = (
        aps[k]
        for k in ("xb", "x8", "wq8", "wk8", "wv8", "bq", "bk", "bv", "wo", "bo", "y")
    )
    xb_r = xb.rearrange("(cc p) s -> p cc s", p=P)
    y_r = y.rearrange("(cc p) s -> p cc s", p=P)
    x8_r = x8.rearrange("(j i p) s -> p j i s", p=P, i=2)
    wq8_r = wq8.rearrange("(j i p) d -> p j i d", p=P, i=2)
    wk8_r = wk8.rearrange("(j i p) d -> p j i d", p=P, i=2)
    wv8_r = wv8.rearrange("(j i p) d -> p j i d", p=P, i=2)
    wo_r = wo.rearrange("(dc p) c -> p dc c", p=P)
    bq_r = bq.rearrange("(dc p) -> p dc", p=P)
    bk_r = bk.rearrange("(dc p) -> p dc", p=P)
    bo_r = bo.rearrange("(cc p) -> p cc", p=P)

    with contextlib.ExitStack() as ctx:
        singles = ctx.enter_context(tc.tile_pool(name="singles", bufs=1))
        etpool = ctx.enter_context(tc.tile_pool(name="et", bufs=4))
        rbpool = ctx.enter_context(tc.tile_pool(name="rb", bufs=4))
        tmppool = ctx.enter_context(tc.tile_pool(name="tmp", bufs=4))
        # PSUM (8 banks): energy 2x2 (rotation), slot A 2, slot B 2.
        pse = ctx.enter_context(tc.tile_pool(name="pse", bufs=2, space="PSUM"))
        psA = ctx.enter_context(tc.tile_pool(name="psA", bufs=1, space="PSUM"))
        psB = ctx.enter_context(tc.tile_pool(name="psB", bufs=1, space="PSUM"))

        # ---- input DMAs ----
        xb_sb = singles.tile([P, CC, S], F16)
        x8_sb = singles.tile([P, 2, 2, S], E4)
        wq8_sb = singles.tile([P, 2, 2, C], E4)
        wk8_sb = singles.tile([P, 2, 2, C], E4)
        wv8_sb = singles.tile([P, 2, 2, C], E4)
        bq_sb = singles.tile([P, CC], F32)
        bk_sb = singles.tile([P, CC], F32)
        bo_sb = singles.tile([P, CC], F32)
        bv_sb = singles.tile([1, C], F16)
        wo_sb = singles.tile([P, CC, C], F16)

        nc.sync.dma_start(out=x8_sb, in_=x8_r)
        nc.sync.dma_start(out=xb_sb[:, 0], in_=xb_r[:, 0])
        nc.sync.dma_start(out=xb_sb[:, 1], in_=xb_r[:, 1])
        nc.scalar.dma_start(out=bq_sb, in_=bq_r)
        nc.scalar.dma_start(out=bk_sb, in_=bk_r)
        for tn in range(CC):
            dsl = slice(tn * P, (tn + 1) * P)
            nc.scalar.dma_start(out=wq8_sb[:, :, :, dsl], in_=wq8_r[:, :, :, dsl])
            nc.scalar.dma_start(out=wk8_sb[:, :, :, dsl], in_=wk8_r[:, :, :, dsl])
        nc.gpsimd.dma_start(out=bv_sb, in_=bv[None, :])
        nc.gpsimd.dma_start(out=wv8_sb, in_=wv8_r)
        nc.gpsimd.dma_start(out=bo_sb, in_=bo_r)
        nc.gpsimd.dma_start(out=wo_sb, in_=wo_r)
        nc.gpsimd.dma_start(out=xb_sb[:, 2], in_=xb_r[:, 2])
        nc.gpsimd.dma_start(out=xb_sb[:, 3], in_=xb_r[:, 3])

        # V' stationary for the DoubleRow AV: per (pair, head, slot):
        # 128 cols = [ones(64) | V(64)] e4m3.
        Vp8 = singles.tile([P, NP, NH, 2, P], E4)
        for pp in range(NP):
            nc.gpsimd.memset(Vp8[:, pp, :, :, 0:64].bitcast(U8), ONE_E4)
        bv_rep = singles.tile([P, C], F16)
        nc.gpsimd.partition_broadcast(bv_rep, bv_sb, channels=P)

        negg_sb = singles.tile([P, 1], F32)
        nc.vector.memset(negg_sb, -G_OFF)

        warm = singles.tile([P, 512], F16)
        nc.vector.memset(warm.bitcast(U16), 0)
        ps_w = psB.tile([P, 2, 512], F32, tag="b")
        for _ in range(N_WARM):
            nc.tensor.matmul(ps_w[:, 0], warm[:, 0:128], warm)

        OTs = [singles.tile([P, S], F16, tag=f"ot{t}", name=f"ot{t}") for t in range(CC)]

        # ---- prologue: all QT/KT via DoubleRow, copies on ACT(q)/DVE(k) ----
        qts = [singles.tile([P, S], F16, name=f"qt{t}") for t in range(CC)]
        kts = [singles.tile([P, S], F16, name=f"kt{t}") for t in range(CC)]
        for i in range(NI):  # i-major: x8 s-half 0 lands first
            sl = slice(i * 512, (i + 1) * 512)
            for tn in range(CC):
                dsl = slice(tn * P, (tn + 1) * P)
                ps_p = pse.tile([P, 2, 512], F32, tag="e")
                for j in range(2):
                    nc.tensor.matmul(
                        ps_p[:, 0], wq8_sb[:, j, :, dsl], x8_sb[:, j, :, sl],
                        start=(j == 0), stop=(j == 1), perf_mode=DR,
                    )
                    nc.tensor.matmul(
                        ps_p[:, 1], wk8_sb[:, j, :, dsl], x8_sb[:, j, :, sl],
                        start=(j == 0), stop=(j == 1), perf_mode=DR,
                    )
                nc.scalar.activation(
                    out=qts[tn][:, sl], in_=ps_p[:, 0], func=IDENT,
                    bias=bq_sb[:, tn : tn + 1], scale=1.0,
                )
                nc.vector.tensor_scalar_add(
                    kts[tn][:, sl], ps_p[:, 1], bk_sb[:, tn : tn + 1]
                )

        def emit_v_projection_chunk(sc):
            ps_v = pse.tile([P, 2, 512], F32, tag="e")
            for j in range(2):
                nc.tensor.matmul(
                    ps_v[:, 0],
                    x8_sb[:, j, :, sc * P : (sc + 1) * P],
                    wv8_sb[:, j],
                    start=(j == 0),
                    stop=(j == 1),
                    perf_mode=DR,
                )
            psv_r = ps_v[:, 0].rearrange("p (h d) -> p h d", h=NH)
            bv_r2 = bv_rep.rearrange("p (h d) -> p h d", h=NH)
            nc.vector.tensor_tensor(
                Vp8[:, sc // 2, :, sc % 2, 64:128], psv_r, bv_r2, ADD
            )

        pending_norm = []
        pending_av = []

        def flush_av(depth=0):
            while len(pending_av) > depth:
                pending_av.pop(0)()

        def flush_norm():
            while pending_norm:
                pending_norm.pop(0)()

        for t in range(CC):
            qt, kt = qts[t], kts[t]
            h0_, h1_ = 2 * t, 2 * t + 1
            for i in range(NI):
                h = 2 * t + i
                sl = slice(i * 512, (i + 1) * 512)
                ps_av = None
                et_pair = None
                for jc in range(SC):
                    k0 = kt[0:64, jc * P : (jc + 1) * P]
                    k1 = kt[64:128, jc * P : (jc + 1) * P]
                    ps_e = pse.tile([P, 2, 512], F32, tag="e")  # head-major
                    nc.tensor.matmul(ps_e[:, 0], k0, qt[0:64, sl])
                    nc.tensor.matmul(ps_e[:, 1], k1, qt[64:128, sl])
                    if t == 0 and i == 0:
                        for sc in {2: (0,), 3: (1,), 4: (2,), 5: (3,),
                                   6: (4, 5), 7: (6, 7)}.get(jc, ()):
                            emit_v_projection_chunk(sc)
                    if jc % 2 == 0:
                        et_pair = etpool.tile([P, 2, 2, 512], E5, tag="et")
                    et_out = et_pair[:, jc % 2]
                    if jc in _DVE_UNITS:
                        nc.vector.tensor_scalar(
                            et_out.bitcast(U8), ps_e, SCH_A, SCH_B, MULT, ADD
                        )
                    else:
                        nc.scalar.activation(
                            out=et_out, in_=ps_e, func=EXP,
                            scale=softmax_scale, bias=negg_sb[:, 0:1],
                        )
                    if jc == 0:
                        ps_av = (psA if h % 2 == 0 else psB).tile(
                            [P, 2, 512], F32, tag="a" if h % 2 == 0 else "b"
                        )
                    if jc == 1:
                        flush_av()  # previous half's last pairs
                        if h == 7:
                            flush_norm()  # norm(h6) early: keep it off the tail
                    if jc == 6:
                        flush_norm()  # norm(h-1), behind this half's DVE exps
                    if jc % 2 == 1:
                        flush_av(depth=1)
                        pp = jc // 2

                        def av(ps_av=ps_av, pp=pp, et=et_pair, ha=h0_, hb=h1_):
                            nc.tensor.matmul(
                                ps_av[:, 0], Vp8[:, pp, ha], et[:, :, 0],
                                start=(pp == 0), stop=(pp == NP - 1),
                                perf_mode=DR,
                            )
                            nc.tensor.matmul(
                                ps_av[:, 1], Vp8[:, pp, hb], et[:, :, 1],
                                start=(pp == 0), stop=(pp == NP - 1),
                                perf_mode=DR,
                            )

                        pending_av.append(av)

                def norm(t=t, sl=sl, ps_av=ps_av):
                    rb = rbpool.tile([64, 2, 512], F32, tag="rb")
                    nc.vector.reciprocal_approx_fast(out=rb, in_=ps_av[0:64])
                    nc.vector.tensor_tensor(
                        OTs[t][0:64, sl], ps_av[64:128, 0], rb[:, 0], MULT
                    )
                    nc.vector.tensor_tensor(
                        OTs[t][64:128, sl], ps_av[64:128, 1], rb[:, 1], MULT
                    )

                pending_norm.append(norm)

        # ---- final projection + bias + residual (fp16) ----
        # cc0/cc1 in the pse rotation (drained by the last exps), cc2 in slot
        # A (free), cc3 in slot B (read by the last norm until ~tail+3us, so
        # its accumulation starts after 18 cover matmuls).
        ps_fs = [
            pse.tile([P, 2, 512], F32, tag="e", name="psf0"),
            pse.tile([P, 2, 512], F32, tag="e", name="psf1"),
            psA.tile([P, 2, 512], F32, tag="a", name="psf2"),
            psB.tile([P, 2, 512], F32, tag="b", name="psf3"),
        ]

        def fmm(dc, cc, start, stop):
            wo_sl = wo_sb[:, dc, cc * P : (cc + 1) * P]
            for i in range(NI):
                sl = slice(i * 512, (i + 1) * 512)
                nc.tensor.matmul(
                    ps_fs[cc][:, i], wo_sl, OTs[dc][:, sl], start=start, stop=stop,
                )

        fmm(0, 0, True, False)
        flush_av()
        fmm(1, 0, False, False)
        flush_norm()  # norm(h7)
        fmm(2, 0, False, False)
        for cc in range(1, CC - 1):
            for dc in range(CC - 1):
                fmm(dc, cc, dc == 0, False)
        for dc in range(CC - 1):
            fmm(dc, 3, dc == 0, False)
        out_q = [nc.sync, nc.scalar, nc.gpsimd]
        for cc in range(CC):
            fmm(CC - 1, cc, False, True)
            for i in range(NI):
                sl = slice(i * 512, (i + 1) * 512)
                tmp = tmppool.tile([P, 512], F16, tag="tmp")
                nc.vector.scalar_tensor_tensor(
                    out=tmp,
                    in0=ps_fs[cc][:, i],
                    scalar=bo_sb[:, cc : cc + 1],
                    in1=xb_sb[:, cc, sl],
                    op0=ADD,
                    op1=ADD,
                )
                out_q[(2 * cc + i) % 3].dma_start(out=y_r[:, cc, sl], in_=tmp)


_NC_CACHE = {}


def _build():
    key = (G_OFF, SCHRAUD_C, N_WARM, _DVE_UNITS)
    if key in _NC_CACHE:
        return _NC_CACHE[key]
    import concourse.bacc as bacc
    import concourse.mybir as mybir
    import concourse.tile as tile

    F32 = mybir.dt.float32
    F16 = mybir.dt.float16
    E4 = mybir.dt.float8e4
    nc = bacc.Bacc("TRN2", target_bir_lowering=False, debug=False)
    aps = {}
    aps["xb"] = nc.dram_tensor("xb", (C, S), F16, kind="ExternalInput").ap()
    aps["x8"] = nc.dram_tensor("x8", (C, S), E4, kind="ExternalInput").ap()
    aps["wq8"] = nc.dram_tensor("wq8", (C, C), E4, kind="ExternalInput").ap()
    aps["wk8"] = nc.dram_tensor("wk8", (C, C), E4, kind="ExternalInput").ap()
    aps["wv8"] = nc.dram_tensor("wv8", (C, C), E4, kind="ExternalInput").ap()
    aps["wo"] = nc.dram_tensor("wo", (C, C), F16, kind="ExternalInput").ap()
    for name in ("bq", "bk", "bo"):
        aps[name] = nc.dram_tensor(name, (C,), F32, kind="ExternalInput").ap()
    aps["bv"] = nc.dram_tensor("bv", (C,), F16, kind="ExternalInput").ap()
    aps["y"] = nc.dram_tensor("y", (C, S), F16, kind="ExternalOutput").ap()
    with tile.TileContext(nc) as tc:
        _emit(nc, tc, mybir, aps)
    nc.compile()
    _NC_CACHE[key] = nc
    return nc


def prepare_in_maps(x, wq, bq, wk, bk, wv, bv, wo, bo):
    """Host-side prep: cast to f16/e4m3, shard x per core."""
    import ml_dtypes

    E4 = ml_dtypes.float8_e4m3
    x = np.asarray(x, dtype=np.float32).reshape(B, C, S)
    weights = {
        "wq8": np.ascontiguousarray(np.asarray(wq, dtype=np.float32).astype(E4)),
        "bq": np.ascontiguousarray(np.asarray(bq, dtype=np.float32)),
        "wk8": np.ascontiguousarray(np.asarray(wk, dtype=np.float32).astype(E4)),
        "bk": np.ascontiguousarray(np.asarray(bk, dtype=np.float32)),
        "wv8": np.ascontiguousarray(np.asarray(wv, dtype=np.float32).astype(E4)),
        "bv": np.ascontiguousarray(np.asarray(bv, dtype=np.float16)),
        "wo": np.ascontiguousarray(np.asarray(wo, dtype=np.float16)),
        "bo": np.ascontiguousarray(np.asarray(bo, dtype=np.float32)),
    }
    return [
        {
            "xb": np.ascontiguousarray(x[b].astype(np.float16)),
            "x8": np.ascontiguousarray(x[b].astype(E4)),
            **weights,
        }
        for b in range(B)
    ]


def kernel(x, wq, bq, wk, bk, wv, bv, wo, bo):
    from concourse import bass_utils

    nc = _build()
    in_maps = prepare_in_maps(x, wq, bq, wk, bk, wv, bv, wo, bo)
    res = bass_utils.run_bass_kernel_spmd(nc, in_maps, core_ids=list(range(B)))
    out = np.stack([r["y"].astype(np.float32) for r in res.results])
    return out.reshape(B, C, 32, 32)


# revision 12
# speedup vs baseline: 1.1574x; 1.0461x over previous
"""AttentionBlock Trainium2 kernel (v6).

Data-parallel: one batch element per NeuronCore (8 cores, no collectives).

Per core, with xr = x[b] viewed as [C, S] (C=512 channels, S=1024 tokens):
    QT = wq^T @ xr + bq   -> [D, S]  (d on partitions; head h = rows 64h..64h+63)
    KT = wk^T @ xr + bk   -> [D, S]
    V  = xr^T @ wv + bv   -> [S, D]  (tokens on partitions)
    per head h: ET[j, i] = KT_h^T . QT_h            (keys j on psum partitions)
                E = exp(ET / sqrt(C) - G)           (G cancels in O'/Z; keeps E
                                                     inside e5m2 range, e <= 11)
                O'T[d, i] = sum_j V[j, d] E[j, i];  Z[i] = sum_j E[j, i]
                OT[d, i] = O'T[d, i] / Z[i]
    y = wo^T @ OT + bo + xr   -> [C, S]

PE modes (measured): fp8 DoubleRow = 2x fp16 FLOP rate; every matmul here is
N=512 at 216ns issue. QKV projections + AV in DR fp8 (et e5m2 x Vp e4m3,
mixed-dtype DR verified); energy fp16 K=64 pairs; final projection fp16.

The schedule is paced by energy-psum drains (exp ~1.1-1.2us per tile vs
0.43us to produce one): v6 gives the rotation THREE 2-bank tiles so a drain
has ~2 full jc of slack, and fills every known latency hole:
  - PSUM: energy 3x2 banks, AV slot A 1x2, hoist slot B 1x2.
  - All AV pairs of a half are deferred into ONE 8-matmul burst at the next
    half's jc1 - PE filler right where the trailing exps would otherwise
    expose their latency. Slot A alternates [AV burst of h] -> [norm(h) reads]
    -> [AV burst of h+1] half by half.
  - norm(h-1) is split: recip flushed at jc4 (between DVE exps 3 and 5),
    mults at jc6 - the DVE queue never delays a rotation drain by more than
    ~0.3us, and slot A's reads finish before the next burst.
  - QT/KT for the i=0 halves are built in a prologue (DR fp8, copies q->ACT,
    k->DVE); each (t, i0) half hoists its own (t, i1) QK into slot B at
    jc6/jc7. x8 ships as two contiguous s-halves so the prologue starts as
    soon as half the activations land.
  - V projection (DR fp8 from x8/wv8) staggers through (t0, i0); its AV
    consumers only pop at (t0, i1) jc1, so the Vp8 copies have a full half of
    slack. wv8 ships split across the scalar and gpsimd queues.
"""

import math
import os

import numpy as np

B = 8
C = 512
S = 1024  # 32*32 tokens
NH = 8
HD = 64
P = 128
CC = C // P  # 4 contraction chunks of 128
NI = 2  # S split into 2 chunks of 512 for matmul free dim
SC = S // P  # 8 key chunks of 128
NP = SC // 2  # 4 key-chunk pairs for the DoubleRow AV

G_OFF = float(os.environ.get("ATTN_G", "1.5"))
SCHRAUD_C = float(os.environ.get("ATTN_SCHRAUD_C", "0.5"))
N_WARM = int(os.environ.get("ATTN_WARM", "6"))
_DVE_UNITS = tuple(
    int(u) for u in os.environ.get("ATTN_DVE_EXP_UNITS", "1,3,5").split(",") if u != ""
)


def _emit(nc, tc, mybir, aps):
    import contextlib

    F32 = mybir.dt.float32
    F16 = mybir.dt.float16
    E4 = mybir.dt.float8e4
    E5 = mybir.dt.float8e5
    U8 = mybir.dt.uint8
    U16 = mybir.dt.uint16
    DR = mybir.MatmulPerfMode.DoubleRow
    MULT = mybir.AluOpType.mult
    ADD = mybir.AluOpType.add
    EXP = mybir.ActivationFunctionType.Exp
    IDENT = mybir.ActivationFunctionType.Identity
    softmax_scale = 1.0 / math.sqrt(C)
    ONE_E4 = 0x38  # 1.0 in e4m3

    SCH_A = softmax_scale * 4.0 / math.log(2.0)
    SCH_B = 15.0 * 4.0 - SCHRAUD_C - G_OFF * 4.0 / math.log(2.0)

    xb, x8, wq8, wk8, wv8, bq, bk, bv, wo, bo, y = (
        aps[k]
        for k in ("xb", "x8", "wq8", "wk8", "wv8", "bq", "bk", "bv", "wo", "bo", "y")
    )
    xb_r = xb.rearrange("(cc p) s -> p cc s", p=P)
    y_r = y.rearrange("(cc p) s -> p cc s", p=P)
    # x8 ships as (sh, C, 512): two contiguous s-halves
    x8_r = x8.rearrange("sh (j i p) s -> p j i sh s", p=P, i=2)
    wq8_r = wq8.rearrange("(j i p) d -> p j i d", p=P, i=2)
    wk8_r = wk8.rearrange("(j i p) d -> p j i d", p=P, i=2)
    wv8_r = wv8.rearrange("(j i p) d -> p j i d", p=P, i=2)
    wo_r = wo.rearrange("(dc p) c -> p dc c", p=P)
    bq_r = bq.rearrange("(dc p) -> p dc", p=P)
    bk_r = bk.rearrange("(dc p) -> p dc", p=P)
    bo_r = bo.rearrange("(cc p) -> p cc", p=P)

    with contextlib.ExitStack() as ctx:
        singles = ctx.enter_context(tc.tile_pool(name="singles", bufs=1))
        etpool = ctx.enter_context(tc.tile_pool(name="et", bufs=6))
        rbpool = ctx.enter_context(tc.tile_pool(name="rb", bufs=2))
        tmppool = ctx.enter_context(tc.tile_pool(name="tmp", bufs=4))
        # PSUM (8 banks): energy/hoist 3x2 (rotation), AV slot A 1x2.
        pse = ctx.enter_context(tc.tile_pool(name="pse", bufs=3, space="PSUM"))
        psA = ctx.enter_context(tc.tile_pool(name="psA", bufs=1, space="PSUM"))

        # ---- input DMAs ----
        xb_sb = singles.tile([P, CC, S], F16)
        x8_sb0 = singles.tile([P, 2, 2, 512], E4)
        x8_sb1 = singles.tile([P, 2, 2, 512], E4)
        wq8_sb = singles.tile([P, 2, 2, C], E4)
        wk8_sb = singles.tile([P, 2, 2, C], E4)
        wv8_sb = singles.tile([P, 2, 2, C], E4)
        bq_sb = singles.tile([P, CC], F32)
        bk_sb = singles.tile([P, CC], F32)
        bo_sb = singles.tile([P, CC], F32)
        bv_sb = singles.tile([1, C], F16)
        wo_sb = singles.tile([P, CC, C], F16)

        x8_sbs = [x8_sb0, x8_sb1]
        nc.sync.dma_start(out=x8_sb0, in_=x8_r[:, :, :, 0])
        nc.sync.dma_start(out=x8_sb1, in_=x8_r[:, :, :, 1])
        nc.sync.dma_start(out=xb_sb[:, 0], in_=xb_r[:, 0])
        nc.sync.dma_start(out=xb_sb[:, 1], in_=xb_r[:, 1])
        nc.scalar.dma_start(out=bq_sb, in_=bq_r)
        nc.scalar.dma_start(out=bk_sb, in_=bk_r)
        for tn in range(CC):
            dsl = slice(tn * P, (tn + 1) * P)
            nc.scalar.dma_start(out=wq8_sb[:, :, :, dsl], in_=wq8_r[:, :, :, dsl])
        nc.scalar.dma_start(out=wv8_sb[:, 0], in_=wv8_r[:, 0])
        nc.gpsimd.dma_start(out=bv_sb, in_=bv[None, :])
        for tn in range(CC):
            dsl = slice(tn * P, (tn + 1) * P)
            nc.gpsimd.dma_start(out=wk8_sb[:, :, :, dsl], in_=wk8_r[:, :, :, dsl])
        nc.gpsimd.dma_start(out=wv8_sb[:, 1], in_=wv8_r[:, 1])
        nc.gpsimd.dma_start(out=bo_sb, in_=bo_r)
        nc.gpsimd.dma_start(out=wo_sb, in_=wo_r)
        nc.gpsimd.dma_start(out=xb_sb[:, 2], in_=xb_r[:, 2])
        nc.gpsimd.dma_start(out=xb_sb[:, 3], in_=xb_r[:, 3])

        # V' stationary for the DoubleRow AV
        Vp8 = singles.tile([P, NP, NH, 2, P], E4)
        for pp in range(NP):
            nc.gpsimd.memset(Vp8[:, pp, :, :, 0:64].bitcast(U8), ONE_E4)
        bv_rep = singles.tile([P, C], F16)
        nc.gpsimd.partition_broadcast(bv_rep, bv_sb, channels=P)

        negg_sb = singles.tile([P, 1], F32)
        nc.vector.memset(negg_sb, -G_OFF)

        warm = singles.tile([P, 512], F16)
        nc.vector.memset(warm.bitcast(U16), 0)
        ps_w = psA.tile([P, 2, 512], F32, tag="a")
        for _ in range(N_WARM):
            nc.tensor.matmul(ps_w[:, 0], warm[:, 0:128], warm)

        OTs = [singles.tile([P, S], F16, tag=f"ot{t}", name=f"ot{t}") for t in range(CC)]

        # ---- prologue: QT/KT for i=0 halves (copies: q->ACT, k->DVE) ----
        qts = [singles.tile([P, S], F16, name=f"qt{t}") for t in range(CC)]
        kts = [singles.tile([P, S], F16, name=f"kt{t}") for t in range(CC)]

        def emit_qk(tn, i, pool, tag, do_q=True, do_k=True):
            dsl = slice(tn * P, (tn + 1) * P)
            ps_p = pool.tile([P, 2, 512], F32, tag=tag)
            for j in range(2):
                if do_q:
                    nc.tensor.matmul(
                        ps_p[:, 0], wq8_sb[:, j, :, dsl], x8_sbs[i][:, j],
                        start=(j == 0), stop=(j == 1), perf_mode=DR,
                    )
                if do_k:
                    nc.tensor.matmul(
                        ps_p[:, 1], wk8_sb[:, j, :, dsl], x8_sbs[i][:, j],
                        start=(j == 0), stop=(j == 1), perf_mode=DR,
                    )
            return ps_p

        def emit_qk_copies(tn, i, ps_p, do_q=True, do_k=True):
            sl = slice(i * 512, (i + 1) * 512)
            if do_q:
                nc.scalar.activation(
                    out=qts[tn][:, sl], in_=ps_p[:, 0], func=IDENT,
                    bias=bq_sb[:, tn : tn + 1], scale=1.0,
                )
            if do_k:
                nc.vector.tensor_scalar_add(
                    kts[tn][:, sl], ps_p[:, 1], bk_sb[:, tn : tn + 1]
                )

        def emit_v_projection_chunk(sc):
            ps_v = pse.tile([P, 2, 512], F32, tag="e")
            for j in range(2):
                nc.tensor.matmul(
                    ps_v[:, 0],
                    x8_sbs[sc // 4][:, j, :, (sc % 4) * P : (sc % 4 + 1) * P],
                    wv8_sb[:, j],
                    start=(j == 0),
                    stop=(j == 1),
                    perf_mode=DR,
                )
            psv_r = ps_v[:, 0].rearrange("p (h d) -> p h d", h=NH)
            bv_r2 = bv_rep.rearrange("p (h d) -> p h d", h=NH)
            nc.vector.tensor_tensor(
                Vp8[:, sc // 2, :, sc % 2, 64:128], psv_r, bv_r2, ADD
            )

        # phase A: q + k first s-half (x8 sh0) for all pairs; vproj sc0-3
        # (tokens 0..511, also sh0) fills the gap until x8 sh1 lands; phase B:
        # k second s-half.
        for tn in range(CC):
            ps_p = emit_qk(tn, 0, pse, "e")
            emit_qk_copies(tn, 0, ps_p)
        for sc in range(4):
            emit_v_projection_chunk(sc)
        for tn in range(CC):
            ps_p = emit_qk(tn, 1, pse, "e", do_q=False)
            emit_qk_copies(tn, 1, ps_p, do_q=False)

        pending_norm = []  # list of (recip_fn, mults_fn)
        recip_done = []
        pending_av = []

        def flush_av():
            while pending_av:
                pending_av.pop(0)()

        def flush_recip():
            if pending_norm:
                r, m = pending_norm.pop(0)
                r()
                recip_done.append(m)

        def flush_mults():
            while recip_done:
                recip_done.pop(0)()

        def flush_norm_all():
            while pending_norm or recip_done:
                flush_recip()
                flush_mults()

        hoist_ps = [None]

        for t in range(CC):
            qt, kt = qts[t], kts[t]
            ha, hb = 2 * t, 2 * t + 1
            for i in range(NI):
                h = 2 * t + i
                sl = slice(i * 512, (i + 1) * 512)
                ps_av = None
                et_pair = None
                for jc in range(SC):
                    k0 = kt[0:64, jc * P : (jc + 1) * P]
                    k1 = kt[64:128, jc * P : (jc + 1) * P]
                    ps_e = pse.tile([P, 2, 512], F32, tag="e")  # head-major
                    nc.tensor.matmul(ps_e[:, 0], k0, qt[0:64, sl])
                    nc.tensor.matmul(ps_e[:, 1], k1, qt[64:128, sl])
                    if t == 0 and i == 0:
                        for sc in {2: (4,), 3: (5,), 4: (6,), 5: (7,)}.get(jc, ()):
                            emit_v_projection_chunk(sc)
                    if i == 0:
                        # hoist this pair's i=1 Q into the pse rotation
                        if jc == 6:
                            hoist_ps[0] = emit_qk(t, 1, pse, "e", do_k=False)
                        elif jc == 7:
                            emit_qk_copies(t, 1, hoist_ps[0], do_k=False)
                    if jc % 2 == 0:
                        et_pair = etpool.tile([P, 2, 2, 512], E5, tag="et")
                    et_out = et_pair[:, jc % 2]
                    if jc in _DVE_UNITS:
                        nc.vector.tensor_scalar(
                            et_out.bitcast(U8), ps_e, SCH_A, SCH_B, MULT, ADD
                        )
                    else:
                        nc.scalar.activation(
                            out=et_out, in_=ps_e, func=EXP,
                            scale=softmax_scale, bias=negg_sb[:, 0:1],
                        )
                    if jc == 0:
                        ps_av = psA.tile([P, 2, 512], F32, tag="a")
                    if jc == 1:
                        flush_av()  # previous half's 4 pairs, one burst
                    if jc == 4:
                        flush_recip()  # norm(h-1) reciprocal
                    if jc == 6:
                        flush_mults()  # norm(h-1) multiplies
                    if jc % 2 == 1:
                        pp = jc // 2

                        def av(ps_av=ps_av, pp=pp, et=et_pair, ha=ha, hb=hb):
                            nc.tensor.matmul(
                                ps_av[:, 0], Vp8[:, pp, ha], et[:, :, 0],
                                start=(pp == 0), stop=(pp == NP - 1),
                                perf_mode=DR,
                            )
                            nc.tensor.matmul(
                                ps_av[:, 1], Vp8[:, pp, hb], et[:, :, 1],
                                start=(pp == 0), stop=(pp == NP - 1),
                                perf_mode=DR,
                            )

                        pending_av.append(av)

                def norm_recip(ps_av=ps_av):
                    rb = rbpool.tile([64, 2, 512], F32, tag="rb")
                    nc.vector.reciprocal_approx_fast(out=rb, in_=ps_av[0:64])
                    return rb

                rb_box = []

                def recip_fn(rb_box=rb_box, ps_av=ps_av):
                    rb_box.append(norm_recip(ps_av))

                def mults_fn(t=t, sl=sl, ps_av=ps_av, rb_box=rb_box):
                    rb = rb_box[0]
                    nc.vector.tensor_tensor(
                        OTs[t][0:64, sl], ps_av[64:128, 0], rb[:, 0], MULT
                    )
                    nc.vector.tensor_tensor(
                        OTs[t][64:128, sl], ps_av[64:128, 1], rb[:, 1], MULT
                    )

                pending_norm.append((recip_fn, mults_fn))

        # ---- final projection + bias + residual (fp16) ----
        ps_fs = [
            pse.tile([P, 2, 512], F32, tag="e", name="psf0"),
            pse.tile([P, 2, 512], F32, tag="e", name="psf1"),
            pse.tile([P, 2, 512], F32, tag="e", name="psf2"),
            psA.tile([P, 2, 512], F32, tag="a", name="psf3"),
        ]

        def fmm(dc, cc, start, stop):
            wo_sl = wo_sb[:, dc, cc * P : (cc + 1) * P]
            for i in range(NI):
                sl = slice(i * 512, (i + 1) * 512)
                nc.tensor.matmul(
                    ps_fs[cc][:, i], wo_sl, OTs[dc][:, sl], start=start, stop=stop,
                )

        fmm(0, 0, True, False)
        flush_av()  # h7's burst
        fmm(1, 0, False, False)
        flush_norm_all()  # norm(h7)
        fmm(2, 0, False, False)
        for cc in range(1, CC - 1):
            for dc in range(CC - 1):
                fmm(dc, cc, dc == 0, False)
        for dc in range(CC - 1):
            fmm(dc, 3, dc == 0, False)
        out_q = [nc.sync, nc.scalar, nc.gpsimd]
        for cc in range(CC):
            fmm(CC - 1, cc, False, True)
            for i in range(NI):
                sl = slice(i * 512, (i + 1) * 512)
                tmp = tmppool.tile([P, 512], F16, tag="tmp")
                nc.vector.scalar_tensor_tensor(
                    out=tmp,
                    in0=ps_fs[cc][:, i],
                    scalar=bo_sb[:, cc : cc + 1],
                    in1=xb_sb[:, cc, sl],
                    op0=ADD,
                    op1=ADD,
                )
                out_q[(2 * cc + i) % 3].dma_start(out=y_r[:, cc, sl], in_=tmp)


_NC_CACHE = {}


def _build():
    key = (G_OFF, SCHRAUD_C, N_WARM, _DVE_UNITS)
    if key in _NC_CACHE:
        return _NC_CACHE[key]
    import concourse.bacc as bacc
    import concourse.mybir as mybir
    import concourse.tile as tile

    F32 = mybir.dt.float32
    F16 = mybir.dt.float16
    E4 = mybir.dt.float8e4
    nc = bacc.Bacc("TRN2", target_bir_lowering=False, debug=False)
    aps = {}
    aps["xb"] = nc.dram_tensor("xb", (C, S), F16, kind="ExternalInput").ap()
    aps["x8"] = nc.dram_tensor("x8", (2, C, 512), E4, kind="ExternalInput").ap()
    aps["wq8"] = nc.dram_tensor("wq8", (C, C), E4, kind="ExternalInput").ap()
    aps["wk8"] = nc.dram_tensor("wk8", (C, C), E4, kind="ExternalInput").ap()
    aps["wv8"] = nc.dram_tensor("wv8", (C, C), E4, kind="ExternalInput").ap()
    aps["wo"] = nc.dram_tensor("wo", (C, C), F16, kind="ExternalInput").ap()
    for name in ("bq", "bk", "bo"):
        aps[name] = nc.dram_tensor(name, (C,), F32, kind="ExternalInput").ap()
    aps["bv"] = nc.dram_tensor("bv", (C,), F16, kind="ExternalInput").ap()
    aps["y"] = nc.dram_tensor("y", (C, S), F16, kind="ExternalOutput").ap()
    with tile.TileContext(nc) as tc:
        _emit(nc, tc, mybir, aps)
    nc.compile()
    _NC_CACHE[key] = nc
    return nc


def prepare_in_maps(x, wq, bq, wk, bk, wv, bv, wo, bo):
    """Host-side prep: cast to f16/e4m3, shard x per core."""
    import ml_dtypes

    E4 = ml_dtypes.float8_e4m3
    x = np.asarray(x, dtype=np.float32).reshape(B, C, S)
    weights = {
        "wq8": np.ascontiguousarray(np.asarray(wq, dtype=np.float32).astype(E4)),
        "bq": np.ascontiguousarray(np.asarray(bq, dtype=np.float32)),
        "wk8": np.ascontiguousarray(np.asarray(wk, dtype=np.float32).astype(E4)),
        "bk": np.ascontiguousarray(np.asarray(bk, dtype=np.float32)),
        "wv8": np.ascontiguousarray(np.asarray(wv, dtype=np.float32).astype(E4)),
        "bv": np.ascontiguousarray(np.asarray(bv, dtype=np.float16)),
        "wo": np.ascontiguousarray(np.asarray(wo, dtype=np.float16)),
        "bo": np.ascontiguousarray(np.asarray(bo, dtype=np.float32)),
    }
    out = []
    for b in range(B):
        x8full = x[b].astype(E4)  # (C, S)
        x8sh = np.stack([x8full[:, 0:512], x8full[:, 512:1024]])  # (2, C, 512)
        out.append(
            {
                "xb": np.ascontiguousarray(x[b].astype(np.float16)),
                "x8": np.ascontiguousarray(x8sh),
                **weights,
            }
        )
    return out


def kernel(x, wq, bq, wk, bk, wv, bv, wo, bo):
    from concourse import bass_utils

    nc = _build()
    in_maps = prepare_in_maps(x, wq, bq, wk, bk, wv, bv, wo, bo)
    res = bass_utils.run_bass_kernel_spmd(nc, in_maps, core_ids=list(range(B)))
    out = np.stack([r["y"].astype(np.float32) for r in res.results])
    return out.reshape(B, C, 32, 32)
